# revision 43
# baseline (speedup 1.0000x reference)
"""Trainium2 Bass kernel for nn_InvariantHeadviaTP.

Reference computation (after dead-code elimination -- y1/y2/gates are never
used by the output):
    x0   = node_vec[:, :128]                  # [N, 128]
    a    = node_embedding                     # [N, 16]
    s0   = einsum('ni,na,iak->nk', x0, a, W1_l0[:, :, :128]) / sqrt(2048) + b1[:128]
    scal = silu(s0)                           # [N, 128]
    mid  = einsum('ni,na,iak->nk', scal, a, W2) / sqrt(2048) + b2   # [N, 16]
    h    = silu(mid @ W3 / 4 + b3)            # [N, 16]
    out  = h @ W4 / 4 + b4                    # [N, 1]

Strategy: data-parallel over 8 cores (2048 nodes each), transposed layout
(features on SBUF partitions, nodes on the free dim).

s0 contraction over c=(i,a) [128*16=2048] is chunked into 16 K=128 chunks
of (32 i's x 4 a's): chunk (q, c4) covers i in [32q, 32q+32), a in
[4c4, 4c4+4).  The moving operand for a chunk is
    u[p, n] = x0rep_q[p, n] * a4rep_c4[p, n]
      x0rep_q[p, n] = x0T[32q + p%32, n]   (x0 rows tiled 4x -> 2MB/core)
      a4rep_c4[p,n] = aT[4c4 + p//32, n]   (a rows repeated 32x -> 2MB/core)
This halves the broadcast DMA vs replicating aT across all 128 partitions
(8MB/core) while keeping all DVE operands SBUF/bf16/contiguous (2x mode).
u is built for q-PAIRS in one DVE op (a4rep read twice via a stride-0 AP).

mid path avoids 16 M=16 matmuls: P[(a,k), n] = sum_i W2[i,a,k] scal[i,n]
via 2 M=128 matmuls, gate-multiply by a16rep (a rows repeated 16x); mid is
linear between the one-hot selector reduction and W3, so both fold into
one lhsT selW3[p, j] = W3[p%16, j]/4 with bias b3' = W3.T b2/4 + b3.

silu is a single scalar-engine activation (HW act table `silu_and_others`;
CoreSim has no Silu LUT -- build with use_silu=False for simulation).
"""

import numpy as np
import ml_dtypes
from contextlib import ExitStack

import concourse.bass as bass
import concourse.bass_isa as bass_isa
import concourse.mybir as mybir
import concourse.tile as tile
from concourse import bacc
from concourse.bass import ts
from concourse.bass_utils import run_bass_kernel_spmd

N_CORES = 8
N_FULL = 16384
NSH = N_FULL // N_CORES          # 2048 nodes per core
A = 16                           # attr dim
M0 = 128                         # MUL0 (scalar channels)
FT = 512                         # matmul moving free size (PSUM bank)
SCALE = 1.0 / np.sqrt(M0 * A)    # path normalization of both fctp einsums
BF16 = ml_dtypes.bfloat16

AF = mybir.ActivationFunctionType
F32 = mybir.dt.float32
DBF16 = mybir.dt.bfloat16

BFP_W2P = 0                      # bfpack column layout
BFP_SELW3 = 2 * M0
BFP_W4 = 2 * M0 + A
BFP_COLS = 2 * M0 + A + 1


def build_nc(nsh: int = NSH, num_devices: int = N_CORES, use_silu: bool = True):
    assert nsh == 4 * FT, "kernel is laid out for 4 PSUM-bank node tiles"

    nc = bacc.Bacc(
        "TRN2",
        target_bir_lowering=False,
        debug=False,
        enable_asserts=False,
        num_devices=num_devices,
    )

    # node data (per-core shard, host-prepped layouts)
    x0rep = nc.dram_tensor("x0rep", [M0, 4 * nsh], DBF16, kind="ExternalInput").ap()
    a4rep = nc.dram_tensor("a4rep", [M0, 4 * nsh], DBF16, kind="ExternalInput").ap()
    a16rep = nc.dram_tensor("a16rep", [M0, 2 * nsh], DBF16, kind="ExternalInput").ap()
    # weights: w0 + one bf16 pack + one f32 pack
    w0 = nc.dram_tensor("w0", [M0, 16 * M0], DBF16, kind="ExternalInput").ap()
    bfpack = nc.dram_tensor("bfpack", [M0, BFP_COLS], DBF16, kind="ExternalInput").ap()
    fpack = nc.dram_tensor("fpack", [M0, 4], F32, kind="ExternalInput").ap()
    outt = nc.dram_tensor("outt", [1, nsh], F32, kind="ExternalOutput").ap()

    with tile.TileContext(nc) as tc, ExitStack() as ctx:
        consts = ctx.enter_context(tc.tile_pool(name="consts", bufs=1))

        HB = nsh // 2  # half-block width: finer DMA granularity at the start
        w0_sb = consts.tile([M0, 16 * M0], DBF16)
        bfpack_sb = consts.tile([M0, BFP_COLS], DBF16)
        fpack_sb = consts.tile([M0, 4], F32)
        x0rep_sb = []
        a4rep_sb = []
        a16rep_sb = []
        for blk in range(8):
            t = consts.tile([M0, HB], DBF16, name=f"x0rep_sb{blk}")
            x0rep_sb.append(t)
        for blk in range(8):
            t = consts.tile([M0, HB], DBF16, name=f"a4rep_sb{blk}")
            a4rep_sb.append(t)
        for blk in range(2):
            t = consts.tile([M0, nsh], DBF16, name=f"a16rep_sb{blk}")
            a16rep_sb.append(t)

        w2p_sb = bfpack_sb[:, BFP_SELW3 - 2 * M0:BFP_SELW3]  # [:, 0:256]
        selw3_sb = bfpack_sb[:, BFP_SELW3:BFP_W4]
        w4_sb = bfpack_sb[0:A, BFP_W4:BFP_W4 + 1]
        b1_sb = fpack_sb[:, 0:1]
        b3p_sb = fpack_sb[0:A, 1:2]
        b4_sb = fpack_sb[0:1, 2:3]
        w4f_sb = fpack_sb[0:A, 3:4]

        # DMA issue costs ~650ns of sequencer time per dma_start, so the
        # issues are spread across the sync and scalar queues (idle at
        # start; scalar's first issue lands after its act-table load).
        # First-use order: chunks iterate c4-outer / q-inner; half-blocks
        # let the first chunk's multiply start ~6us earlier.
        nc.sync.dma_start(a4rep_sb[0][:], a4rep[:, ts(0, HB)])
        nc.scalar.dma_start(a4rep_sb[1][:], a4rep[:, ts(1, HB)])
        nc.sync.dma_start(x0rep_sb[0][:], x0rep[:, ts(0, HB)])
        nc.scalar.dma_start(x0rep_sb[1][:], x0rep[:, ts(1, HB)])
        nc.sync.dma_start(w0_sb[:], w0)
        nc.scalar.dma_start(x0rep_sb[2][:], x0rep[:, ts(2, HB)])
        nc.sync.dma_start(x0rep_sb[3][:], x0rep[:, ts(3, HB)])
        nc.scalar.dma_start(x0rep_sb[4][:], x0rep[:, ts(4, HB)])
        nc.sync.dma_start(x0rep_sb[5][:], x0rep[:, ts(5, HB)])
        nc.scalar.dma_start(a4rep_sb[2][:], a4rep[:, ts(2, HB)])
        nc.sync.dma_start(a4rep_sb[3][:], a4rep[:, ts(3, HB)])
        nc.scalar.dma_start(x0rep_sb[6][:], x0rep[:, ts(6, HB)])
        nc.sync.dma_start(x0rep_sb[7][:], x0rep[:, ts(7, HB)])
        nc.scalar.dma_start(a4rep_sb[4][:], a4rep[:, ts(4, HB)])
        nc.sync.dma_start(a4rep_sb[5][:], a4rep[:, ts(5, HB)])
        nc.scalar.dma_start(a4rep_sb[6][:], a4rep[:, ts(6, HB)])
        nc.sync.dma_start(a4rep_sb[7][:], a4rep[:, ts(7, HB)])
        nc.scalar.dma_start(a16rep_sb[0][:], a16rep[:, ts(0, nsh)])
        nc.sync.dma_start(a16rep_sb[1][:], a16rep[:, ts(1, nsh)])
        nc.scalar.dma_start(bfpack_sb[:], bfpack)
        nc.sync.dma_start(fpack_sb[:], fpack)

        u_pool = ctx.enter_context(tc.tile_pool(name="u", bufs=4))
        s_pool = ctx.enter_context(tc.tile_pool(name="s", bufs=2))
        scal_pool = ctx.enter_context(tc.tile_pool(name="scal", bufs=4))
        pm_pool = ctx.enter_context(tc.tile_pool(name="pm", bufs=4))
        o_pool = ctx.enter_context(tc.tile_pool(name="o", bufs=1))
        # PSUM budget (8 banks): s0 4 tags + p 2 tags + mm 2 tags, bufs=1.
        ps_s0 = ctx.enter_context(tc.tile_pool(name="ps_s0", bufs=1, space="PSUM"))
        ps_p = ctx.enter_context(tc.tile_pool(name="ps_p", bufs=1, space="PSUM"))
        ps_mm = ctx.enter_context(tc.tile_pool(name="ps_mm", bufs=1, space="PSUM"))

        ob_all = o_pool.tile([1, nsh], F32)

        # PE warm-up: dummy matmuls with no DMA dependencies, running while
        # the input DMAs stream in. Keeps the tensor engine busy from boot
        # so the DVFS fast window starts before the real matmul stream.
        wu_sb = consts.tile([M0, FT], DBF16, name="wu_sb")
        nc.vector.memset(wu_sb[:], 0.0)
        wu_ps = ps_p.tile([M0, FT], F32, tag="p_0", name="wu_ps")
        for w in range(14):
            nc.tensor.matmul(wu_ps[:], wu_sb[:, 0:M0], wu_sb[:],
                             start=(w == 0), stop=(w == 13))

        # ---- s0 accumulation: 16 chunks of K=128 = (32 i x 4 a), run as
        # two phases over node-halves (hblk) so the first half's epilogue
        # interleaves with the second half's matmul stream. ----
        s0_ps = [
            ps_s0.tile([M0, FT], F32, tag=f"s0_{f}", name=f"s0_{f}")
            for f in range(4)
        ]

        def s0_phase(hblk):
            for c4 in range(4):
                for q in range(4):
                    ci = q * 4 + c4  # w0 host block index
                    u = u_pool.tile([M0, HB], DBF16, tag="u")
                    nc.vector.tensor_mul(
                        u[:], x0rep_sb[2 * q + hblk][:], a4rep_sb[2 * c4 + hblk][:]
                    )
                    for floc in range(2):
                        f = 2 * hblk + floc
                        nc.tensor.matmul(
                            s0_ps[f][:],
                            w0_sb[:, ts(ci, M0)],
                            u[:, ts(floc, FT)],
                            start=(c4 == 0 and q == 0),
                            stop=(c4 == 3 and q == 3),
                        )

        # ---- epilogue in pairs of 512-node tiles; matmuls grouped by
        # lhsT so each weight is loaded once per pair. ----
        def epilogue_steps(g):
            fs = [2 * g, 2 * g + 1]

            scal = {}
            for f in fs:
                sc = scal_pool.tile([M0, FT], DBF16, tag=f"scal_{f % 2}",
                                    name=f"scal_{f}")
                if use_silu:
                    nc.scalar.activation(sc[:], s0_ps[f][:], AF.Silu, bias=b1_sb)
                else:
                    s_sig = s_pool.tile([M0, FT], DBF16, tag="s_sig")
                    nc.scalar.activation(s_sig[:], s0_ps[f][:], AF.Sigmoid,
                                         bias=b1_sb)
                    s_idn = s_pool.tile([M0, FT], DBF16, tag="s_idn")
                    nc.scalar.activation(s_idn[:], s0_ps[f][:], AF.Identity,
                                         bias=b1_sb)
                    nc.vector.tensor_mul(sc[:], s_idn[:], s_sig[:])
                scal[f] = sc
            yield

            # P[(a_l,k), n] for a-halves 0/1, gated by a16rep.
            mm = {}
            for f in fs:
                mm[f] = ps_mm.tile([65, FT], F32, tag=f"mm_{f % 2}", name=f"mm_{f}")
            pm_t = {}
            for ah in range(2):
                for f in fs:
                    p_ps = ps_p.tile([M0, FT], F32, tag=f"p_{f % 2}",
                                     name=f"p_{ah}_{f}")
                    nc.tensor.matmul(
                        p_ps[:], w2p_sb[:, ts(ah, M0)], scal[f][:],
                        start=True, stop=True,
                    )
                    pm = pm_pool.tile([M0, FT], DBF16, tag=f"pm_{ah}_{f % 2}",
                                      name=f"pm_{ah}_{f}")
                    nc.vector.tensor_mul(
                        pm[:], p_ps[:],
                        a16rep_sb[ah][:, ts(f, FT)],
                    )
                    pm_t[(ah, f)] = pm
                yield
            for f in fs:
                for ah in range(2):
                    nc.tensor.matmul(
                        mm[f][32:48, :], selw3_sb, pm_t[(ah, f)][:],
                        start=(ah == 0), stop=(ah == 1),
                    )
            yield

            hb = {}
            for f in fs:
                hbt = s_pool.tile([A, FT], DBF16, tag="hb", name=f"hb_{f}")
                if use_silu:
                    nc.scalar.activation(hbt[:], mm[f][32:48, :], AF.Silu,
                                         bias=b3p_sb)
                else:
                    h_sig = s_pool.tile([A, FT], DBF16, tag="h_sig")
                    nc.scalar.activation(h_sig[:], mm[f][32:48, :], AF.Sigmoid,
                                         bias=b3p_sb)
                    h_idn = s_pool.tile([A, FT], DBF16, tag="h_idn")
                    nc.scalar.activation(h_idn[:], mm[f][32:48, :], AF.Identity,
                                         bias=b3p_sb)
                    nc.vector.tensor_mul(hbt[:], h_idn[:], h_sig[:])
                hb[f] = hbt
            yield
            for f in fs:
                nc.tensor.matmul(mm[f][64:65, :], w4_sb, hb[f][:],
                                 start=True, stop=True)
            for f in fs:
                nc.scalar.activation(ob_all[0:1, ts(f, FT)], mm[f][64:65, :],
                                     AF.Identity, bias=b4_sb)

            eng = nc.sync if g == 0 else nc.scalar
            eng.dma_start(outt[:, bass.ds(g * 2 * FT, 2 * FT)],
                          ob_all[0:1, bass.ds(g * 2 * FT, 2 * FT)])

        def s0_phase_interleaved(hblk, steps):
            ck = 0
            for c4 in range(4):
                for q in range(4):
                    ci = q * 4 + c4
                    u = u_pool.tile([M0, HB], DBF16, tag="u")
                    nc.vector.tensor_mul(
                        u[:], x0rep_sb[2 * q + hblk][:], a4rep_sb[2 * c4 + hblk][:]
                    )
                    for floc in range(2):
                        f = 2 * hblk + floc
                        nc.tensor.matmul(
                            s0_ps[f][:],
                            w0_sb[:, ts(ci, M0)],
                            u[:, ts(floc, FT)],
                            start=(c4 == 0 and q == 0),
                            stop=(c4 == 3 and q == 3),
                        )
                    if ck % 2 == 1:
                        next(steps, None)
                    ck += 1

        s0_phase(0)
        epi0 = epilogue_steps(0)
        s0_phase_interleaved(1, epi0)
        for _ in epi0:
            pass
        for _ in epilogue_steps(1):
            pass

    nc.compile()
    return nc


def prep_host(inputs: dict, nsh: int = NSH, n_cores: int = N_CORES):
    """Host-side prep: slice/transpose/cast inputs, build per-core in_maps."""
    node_vec = np.asarray(inputs["node_vec"], dtype=np.float32)
    node_embedding = np.asarray(inputs["node_embedding"], dtype=np.float32)
    W1_l0 = np.asarray(inputs["W1_l0"], dtype=np.float32)
    b1 = np.asarray(inputs["b1"], dtype=np.float32)
    W2 = np.asarray(inputs["W2"], dtype=np.float32)
    b2 = np.asarray(inputs["b2"], dtype=np.float32)
    W3 = np.asarray(inputs["W3"], dtype=np.float32)
    b3 = np.asarray(inputs["b3"], dtype=np.float32)
    W4 = np.asarray(inputs["W4"], dtype=np.float32)
    b4 = np.asarray(inputs["b4"], dtype=np.float32)

    x0T = np.ascontiguousarray(node_vec[:, :M0].T).astype(BF16)      # [128, N]
    aT = np.ascontiguousarray(node_embedding.T).astype(BF16)         # [16, N]

    # w0 chunk ci = (q, c4): [p, k] = W[32q + p%32, 4c4 + p//32, k]
    W = (W1_l0[:, :, :M0] * SCALE).astype(np.float32)                # [128,16,128]
    w0_blocks = []
    for ci in range(16):
        q, c4 = ci // 4, ci % 4
        blk = W[q * 32:(q + 1) * 32, c4 * 4:(c4 + 1) * 4, :]         # [32, 4, 128]
        w0_blocks.append(blk.transpose(1, 0, 2).reshape(M0, M0))     # p = a_l*32+i_l
    w0h = np.concatenate(w0_blocks, axis=1).astype(BF16)             # [128, 2048]

    w2ph = (W2 * SCALE).reshape(M0, A * A)                           # [128, 256]
    # selector+W3 fold: selW3[p, j] = W3[p%16, j]/4; b3' = W3.T b2/4 + b3
    selw3 = np.tile(W3 / np.sqrt(A), (8, 1))                         # [128, 16]
    b3p = (W3.T @ b2) / np.sqrt(A) + b3                              # [16]
    w4h = W4 / np.sqrt(A)                                            # [16, 1]

    bfpack = np.zeros((M0, BFP_COLS), dtype=np.float32)
    bfpack[:, 0:2 * M0] = w2ph
    bfpack[:, BFP_SELW3:BFP_W4] = selw3
    bfpack[0:A, BFP_W4] = w4h[:, 0]
    bfpack = bfpack.astype(BF16)

    fpack = np.zeros((M0, 4), dtype=np.float32)
    fpack[:, 0] = b1[:M0]
    fpack[0:A, 1] = b3p
    fpack[0, 2] = b4[0]
    fpack[0:A, 3] = w4h[:, 0]

    shared = {"w0": w0h, "bfpack": bfpack, "fpack": fpack}
    in_maps = []
    for c in range(n_cores):
        sl = slice(c * nsh, (c + 1) * nsh)
        x0s = x0T[:, sl]
        ats = aT[:, sl]
        x0rep = np.concatenate(
            [np.tile(x0s[q * 32:(q + 1) * 32, :], (4, 1)) for q in range(4)], axis=1
        )                                                            # [128, 4*nsh]
        a4rep = np.concatenate(
            [np.repeat(ats[c4 * 4:(c4 + 1) * 4, :], 32, axis=0) for c4 in range(4)],
            axis=1,
        )                                                            # [128, 4*nsh]
        a16rep = np.concatenate(
            [np.repeat(ats[ah * 8:(ah + 1) * 8, :], 16, axis=0) for ah in range(2)],
            axis=1,
        )                                                            # [128, 2*nsh]
        in_maps.append({
            "x0rep": np.ascontiguousarray(x0rep),
            "a4rep": np.ascontiguousarray(a4rep),
            "a16rep": np.ascontiguousarray(a16rep),
            **shared,
        })
    return in_maps


_NC_CACHE = {}


def _get_nc():
    if "nc" not in _NC_CACHE:
        _NC_CACHE["nc"] = build_nc()
    return _NC_CACHE["nc"]


def kernel_with_results(trace: bool = False, **inputs):
    nc = _get_nc()
    in_maps = prep_host(inputs)
    res = run_bass_kernel_spmd(
        nc, in_maps, core_ids=list(range(N_CORES)), trace=trace,
    )
    out = np.empty((N_FULL, 1), dtype=np.float32)
    for c in range(N_CORES):
        out[c * NSH:(c + 1) * NSH, 0] = res.results[c]["outt"][0]
    return out, res


def kernel(**inputs) -> np.ndarray:
    out, _ = kernel_with_results(trace=False, **inputs)
    return out


# revision 44
# speedup vs baseline: 1.0255x; 1.0255x over previous
"""Trainium2 Bass kernel for nn_InvariantHeadviaTP.

Reference computation (after dead-code elimination -- y1/y2/gates are never
used by the output):
    x0   = node_vec[:, :128]                  # [N, 128]
    a    = node_embedding                     # [N, 16]
    s0   = einsum('ni,na,iak->nk', x0, a, W1_l0[:, :, :128]) / sqrt(2048) + b1[:128]
    scal = silu(s0)                           # [N, 128]
    mid  = einsum('ni,na,iak->nk', scal, a, W2) / sqrt(2048) + b2   # [N, 16]
    h    = silu(mid @ W3 / 4 + b3)            # [N, 16]
    out  = h @ W4 / 4 + b4                    # [N, 1]

Strategy: data-parallel over 8 cores (2048 nodes each), transposed layout
(features on SBUF partitions, nodes on the free dim).

s0 contraction over c=(i,a) [128*16=2048] is chunked into 16 K=128 chunks
of (32 i's x 4 a's): chunk (q, c4) covers i in [32q, 32q+32), a in
[4c4, 4c4+4).  The moving operand for a chunk is
    u[p, n] = x0rep_q[p, n] * a4rep_c4[p, n]
      x0rep_q[p, n] = x0T[32q + p%32, n]   (x0 rows tiled 4x -> 2MB/core)
      a4rep_c4[p,n] = aT[4c4 + p//32, n]   (a rows repeated 32x -> 2MB/core)
This halves the broadcast DMA vs replicating aT across all 128 partitions
(8MB/core) while keeping all DVE operands SBUF/bf16/contiguous (2x mode).
u is built for q-PAIRS in one DVE op (a4rep read twice via a stride-0 AP).

mid path avoids 16 M=16 matmuls: P[(a,k), n] = sum_i W2[i,a,k] scal[i,n]
via 2 M=128 matmuls, gate-multiply by a16rep (a rows repeated 16x); mid is
linear between the one-hot selector reduction and W3, so both fold into
one lhsT selW3[p, j] = W3[p%16, j]/4 with bias b3' = W3.T b2/4 + b3.

silu is a single scalar-engine activation (HW act table `silu_and_others`;
CoreSim has no Silu LUT -- build with use_silu=False for simulation).
"""

import numpy as np
import ml_dtypes
from contextlib import ExitStack

import concourse.bass as bass
import concourse.bass_isa as bass_isa
import concourse.mybir as mybir
import concourse.tile as tile
from concourse import bacc
from concourse.bass import ts
from concourse.bass_utils import run_bass_kernel_spmd

N_CORES = 8
N_FULL = 16384
NSH = N_FULL // N_CORES          # 2048 nodes per core
A = 16                           # attr dim
M0 = 128                         # MUL0 (scalar channels)
FT = 512                         # matmul moving free size (PSUM bank)
SCALE = 1.0 / np.sqrt(M0 * A)    # path normalization of both fctp einsums
BF16 = ml_dtypes.bfloat16

AF = mybir.ActivationFunctionType
F32 = mybir.dt.float32
DBF16 = mybir.dt.bfloat16

BFP_W2P = 0                      # bfpack column layout
BFP_SELW3 = 2 * M0
BFP_W4 = 2 * M0 + A
BFP_COLS = 2 * M0 + A + 1


def build_nc(nsh: int = NSH, num_devices: int = N_CORES, use_silu: bool = True):
    assert nsh == 4 * FT, "kernel is laid out for 4 PSUM-bank node tiles"

    nc = bacc.Bacc(
        "TRN2",
        target_bir_lowering=False,
        debug=False,
        enable_asserts=False,
        num_devices=num_devices,
    )

    # node data (per-core shard, host-prepped layouts)
    x0rep = nc.dram_tensor("x0rep", [M0, 4 * nsh], DBF16, kind="ExternalInput").ap()
    a4rep = nc.dram_tensor("a4rep", [M0, 4 * nsh], DBF16, kind="ExternalInput").ap()
    a16rep = nc.dram_tensor("a16rep", [M0, 2 * nsh], DBF16, kind="ExternalInput").ap()
    # weights: w0 + one bf16 pack + one f32 pack
    w0 = nc.dram_tensor("w0", [M0, 16 * M0], DBF16, kind="ExternalInput").ap()
    bfpack = nc.dram_tensor("bfpack", [M0, BFP_COLS], DBF16, kind="ExternalInput").ap()
    fpack = nc.dram_tensor("fpack", [M0, 4], F32, kind="ExternalInput").ap()
    outt = nc.dram_tensor("outt", [1, nsh], F32, kind="ExternalOutput").ap()

    with tile.TileContext(nc) as tc, ExitStack() as ctx:
        consts = ctx.enter_context(tc.tile_pool(name="consts", bufs=1))

        HB = nsh // 2  # half-block width: finer DMA granularity at the start
        w0_sb = consts.tile([M0, 16 * M0], DBF16)
        bfpack_sb = consts.tile([M0, BFP_COLS], DBF16)
        fpack_sb = consts.tile([M0, 4], F32)
        x0rep_sb = []
        a4rep_sb = []
        a16rep_sb = []
        for blk in range(8):
            t = consts.tile([M0, HB], DBF16, name=f"x0rep_sb{blk}")
            x0rep_sb.append(t)
        for blk in range(8):
            t = consts.tile([M0, HB], DBF16, name=f"a4rep_sb{blk}")
            a4rep_sb.append(t)
        for blk in range(2):
            t = consts.tile([M0, nsh], DBF16, name=f"a16rep_sb{blk}")
            a16rep_sb.append(t)

        w2p_sb = bfpack_sb[:, BFP_SELW3 - 2 * M0:BFP_SELW3]  # [:, 0:256]
        selw3_sb = bfpack_sb[:, BFP_SELW3:BFP_W4]
        w4_sb = bfpack_sb[0:A, BFP_W4:BFP_W4 + 1]
        b1_sb = fpack_sb[:, 0:1]
        b3p_sb = fpack_sb[0:A, 1:2]
        b4_sb = fpack_sb[0:1, 2:3]
        w4f_sb = fpack_sb[0:A, 3:4]

        # DMA issue costs ~650ns of sequencer time per dma_start, so the
        # issues are spread across the sync and scalar queues (idle at
        # start; scalar's first issue lands after its act-table load).
        # First-use order: chunks iterate c4-outer / q-inner; half-blocks
        # let the first chunk's multiply start ~6us earlier.
        nc.sync.dma_start(a4rep_sb[0][:], a4rep[:, ts(0, HB)])
        nc.scalar.dma_start(a4rep_sb[1][:], a4rep[:, ts(1, HB)])
        nc.sync.dma_start(x0rep_sb[0][:], x0rep[:, ts(0, HB)])
        nc.scalar.dma_start(x0rep_sb[1][:], x0rep[:, ts(1, HB)])
        nc.sync.dma_start(w0_sb[:], w0)
        nc.scalar.dma_start(x0rep_sb[2][:], x0rep[:, ts(2, HB)])
        nc.sync.dma_start(x0rep_sb[3][:], x0rep[:, ts(3, HB)])
        nc.scalar.dma_start(x0rep_sb[4][:], x0rep[:, ts(4, HB)])
        nc.sync.dma_start(x0rep_sb[5][:], x0rep[:, ts(5, HB)])
        nc.scalar.dma_start(a4rep_sb[2][:], a4rep[:, ts(2, HB)])
        nc.sync.dma_start(a4rep_sb[3][:], a4rep[:, ts(3, HB)])
        nc.scalar.dma_start(x0rep_sb[6][:], x0rep[:, ts(6, HB)])
        nc.sync.dma_start(x0rep_sb[7][:], x0rep[:, ts(7, HB)])
        nc.scalar.dma_start(a4rep_sb[4][:], a4rep[:, ts(4, HB)])
        nc.sync.dma_start(a4rep_sb[5][:], a4rep[:, ts(5, HB)])
        nc.scalar.dma_start(a4rep_sb[6][:], a4rep[:, ts(6, HB)])
        nc.sync.dma_start(a4rep_sb[7][:], a4rep[:, ts(7, HB)])
        nc.scalar.dma_start(a16rep_sb[0][:], a16rep[:, ts(0, nsh)])
        nc.sync.dma_start(a16rep_sb[1][:], a16rep[:, ts(1, nsh)])
        nc.scalar.dma_start(bfpack_sb[:], bfpack)
        nc.sync.dma_start(fpack_sb[:], fpack)

        u_pool = ctx.enter_context(tc.tile_pool(name="u", bufs=4))
        s_pool = ctx.enter_context(tc.tile_pool(name="s", bufs=2))
        scal_pool = ctx.enter_context(tc.tile_pool(name="scal", bufs=4))
        pm_pool = ctx.enter_context(tc.tile_pool(name="pm", bufs=4))
        o_pool = ctx.enter_context(tc.tile_pool(name="o", bufs=1))
        # PSUM budget (8 banks): s0 4 tags + p 2 tags + mm 2 tags, bufs=1.
        ps_s0 = ctx.enter_context(tc.tile_pool(name="ps_s0", bufs=1, space="PSUM"))
        ps_p = ctx.enter_context(tc.tile_pool(name="ps_p", bufs=1, space="PSUM"))
        ps_mm = ctx.enter_context(tc.tile_pool(name="ps_mm", bufs=1, space="PSUM"))

        ob_all = o_pool.tile([1, nsh], F32)

        # PE warm-up: dummy matmuls with no DMA dependencies, running while
        # the input DMAs stream in. Keeps the tensor engine busy from boot
        # so the DVFS fast window starts before the real matmul stream.
        wu_sb = consts.tile([M0, FT], DBF16, name="wu_sb")
        nc.vector.memset(wu_sb[:], 0.0)
        wu_ps = ps_p.tile([M0, FT], F32, tag="p_0", name="wu_ps")
        for w in range(14):
            nc.tensor.matmul(wu_ps[:], wu_sb[:, 0:M0], wu_sb[:],
                             start=(w == 0), stop=(w == 13))

        # ---- s0 accumulation: 16 chunks of K=128 = (32 i x 4 a), run as
        # two phases over node-halves (hblk) so the first half's epilogue
        # interleaves with the second half's matmul stream. ----
        s0_ps = [
            ps_s0.tile([M0, FT], F32, tag=f"s0_{f}", name=f"s0_{f}")
            for f in range(4)
        ]

        def s0_phase(hblk):
            for c4 in range(4):
                for q in range(4):
                    ci = q * 4 + c4  # w0 host block index
                    u = u_pool.tile([M0, HB], DBF16, tag="u")
                    nc.vector.tensor_mul(
                        u[:], x0rep_sb[2 * q + hblk][:], a4rep_sb[2 * c4 + hblk][:]
                    )
                    for floc in range(2):
                        f = 2 * hblk + floc
                        nc.tensor.matmul(
                            s0_ps[f][:],
                            w0_sb[:, ts(ci, M0)],
                            u[:, ts(floc, FT)],
                            start=(c4 == 0 and q == 0),
                            stop=(c4 == 3 and q == 3),
                        )

        # ---- epilogue in pairs of 512-node tiles; matmuls grouped by
        # lhsT so each weight is loaded once per pair. ----
        def epilogue_steps(g):
            fs = [2 * g, 2 * g + 1]

            scal = {}
            for f in fs:
                sc = scal_pool.tile([M0, FT], DBF16, tag=f"scal_{f % 2}",
                                    name=f"scal_{f}")
                if use_silu:
                    nc.scalar.activation(sc[:], s0_ps[f][:], AF.Silu, bias=b1_sb)
                else:
                    s_sig = s_pool.tile([M0, FT], DBF16, tag="s_sig")
                    nc.scalar.activation(s_sig[:], s0_ps[f][:], AF.Sigmoid,
                                         bias=b1_sb)
                    s_idn = s_pool.tile([M0, FT], DBF16, tag="s_idn")
                    nc.scalar.activation(s_idn[:], s0_ps[f][:], AF.Identity,
                                         bias=b1_sb)
                    nc.vector.tensor_mul(sc[:], s_idn[:], s_sig[:])
                scal[f] = sc
            yield

            # P[(a_l,k), n] for a-halves 0/1, gated by a16rep.
            mm = {}
            for f in fs:
                mm[f] = ps_mm.tile([65, FT], F32, tag=f"mm_{f % 2}", name=f"mm_{f}")
            pm_t = {}
            for ah in range(2):
                for f in fs:
                    p_ps = ps_p.tile([M0, FT], F32, tag=f"p_{f % 2}",
                                     name=f"p_{ah}_{f}")
                    nc.tensor.matmul(
                        p_ps[:], w2p_sb[:, ts(ah, M0)], scal[f][:],
                        start=True, stop=True,
                    )
                    pm = pm_pool.tile([M0, FT], DBF16, tag=f"pm_{ah}_{f % 2}",
                                      name=f"pm_{ah}_{f}")
                    nc.vector.tensor_mul(
                        pm[:], p_ps[:],
                        a16rep_sb[ah][:, ts(f, FT)],
                    )
                    pm_t[(ah, f)] = pm
                yield
            for f in fs:
                for ah in range(2):
                    nc.tensor.matmul(
                        mm[f][32:48, :], selw3_sb, pm_t[(ah, f)][:],
                        start=(ah == 0), stop=(ah == 1),
                    )
            yield

            hb = {}
            for f in fs:
                hbt = s_pool.tile([A, FT], DBF16, tag="hb", name=f"hb_{f}")
                if use_silu:
                    nc.scalar.activation(hbt[:], mm[f][32:48, :], AF.Silu,
                                         bias=b3p_sb)
                else:
                    h_sig = s_pool.tile([A, FT], DBF16, tag="h_sig")
                    nc.scalar.activation(h_sig[:], mm[f][32:48, :], AF.Sigmoid,
                                         bias=b3p_sb)
                    h_idn = s_pool.tile([A, FT], DBF16, tag="h_idn")
                    nc.scalar.activation(h_idn[:], mm[f][32:48, :], AF.Identity,
                                         bias=b3p_sb)
                    nc.vector.tensor_mul(hbt[:], h_idn[:], h_sig[:])
                hb[f] = hbt
            yield
            for f in fs:
                nc.tensor.matmul(mm[f][64:65, :], w4_sb, hb[f][:],
                                 start=True, stop=True)
            for f in fs:
                nc.scalar.activation(ob_all[0:1, ts(f, FT)], mm[f][64:65, :],
                                     AF.Identity, bias=b4_sb)

            eng = nc.sync if g == 0 else nc.scalar
            eng.dma_start(outt[:, bass.ds(g * 2 * FT, 2 * FT)],
                          ob_all[0:1, bass.ds(g * 2 * FT, 2 * FT)])

        s0_phase(0)
        s0_phase(1)
        for _ in epilogue_steps(0):
            pass
        for _ in epilogue_steps(1):
            pass

    nc.compile()
    return nc


def prep_host(inputs: dict, nsh: int = NSH, n_cores: int = N_CORES):
    """Host-side prep: slice/transpose/cast inputs, build per-core in_maps."""
    node_vec = np.asarray(inputs["node_vec"], dtype=np.float32)
    node_embedding = np.asarray(inputs["node_embedding"], dtype=np.float32)
    W1_l0 = np.asarray(inputs["W1_l0"], dtype=np.float32)
    b1 = np.asarray(inputs["b1"], dtype=np.float32)
    W2 = np.asarray(inputs["W2"], dtype=np.float32)
    b2 = np.asarray(inputs["b2"], dtype=np.float32)
    W3 = np.asarray(inputs["W3"], dtype=np.float32)
    b3 = np.asarray(inputs["b3"], dtype=np.float32)
    W4 = np.asarray(inputs["W4"], dtype=np.float32)
    b4 = np.asarray(inputs["b4"], dtype=np.float32)

    x0T = np.ascontiguousarray(node_vec[:, :M0].T).astype(BF16)      # [128, N]
    aT = np.ascontiguousarray(node_embedding.T).astype(BF16)         # [16, N]

    # w0 chunk ci = (q, c4): [p, k] = W[32q + p%32, 4c4 + p//32, k]
    W = (W1_l0[:, :, :M0] * SCALE).astype(np.float32)                # [128,16,128]
    w0_blocks = []
    for ci in range(16):
        q, c4 = ci // 4, ci % 4
        blk = W[q * 32:(q + 1) * 32, c4 * 4:(c4 + 1) * 4, :]         # [32, 4, 128]
        w0_blocks.append(blk.transpose(1, 0, 2).reshape(M0, M0))     # p = a_l*32+i_l
    w0h = np.concatenate(w0_blocks, axis=1).astype(BF16)             # [128, 2048]

    w2ph = (W2 * SCALE).reshape(M0, A * A)                           # [128, 256]
    # selector+W3 fold: selW3[p, j] = W3[p%16, j]/4; b3' = W3.T b2/4 + b3
    selw3 = np.tile(W3 / np.sqrt(A), (8, 1))                         # [128, 16]
    b3p = (W3.T @ b2) / np.sqrt(A) + b3                              # [16]
    w4h = W4 / np.sqrt(A)                                            # [16, 1]

    bfpack = np.zeros((M0, BFP_COLS), dtype=np.float32)
    bfpack[:, 0:2 * M0] = w2ph
    bfpack[:, BFP_SELW3:BFP_W4] = selw3
    bfpack[0:A, BFP_W4] = w4h[:, 0]
    bfpack = bfpack.astype(BF16)

    fpack = np.zeros((M0, 4), dtype=np.float32)
    fpack[:, 0] = b1[:M0]
    fpack[0:A, 1] = b3p
    fpack[0, 2] = b4[0]
    fpack[0:A, 3] = w4h[:, 0]

    shared = {"w0": w0h, "bfpack": bfpack, "fpack": fpack}
    in_maps = []
    for c in range(n_cores):
        sl = slice(c * nsh, (c + 1) * nsh)
        x0s = x0T[:, sl]
        ats = aT[:, sl]
        x0rep = np.concatenate(
            [np.tile(x0s[q * 32:(q + 1) * 32, :], (4, 1)) for q in range(4)], axis=1
        )                                                            # [128, 4*nsh]
        a4rep = np.concatenate(
            [np.repeat(ats[c4 * 4:(c4 + 1) * 4, :], 32, axis=0) for c4 in range(4)],
            axis=1,
        )                                                            # [128, 4*nsh]
        a16rep = np.concatenate(
            [np.repeat(ats[ah * 8:(ah + 1) * 8, :], 16, axis=0) for ah in range(2)],
            axis=1,
        )                                                            # [128, 2*nsh]
        in_maps.append({
            "x0rep": np.ascontiguousarray(x0rep),
            "a4rep": np.ascontiguousarray(a4rep),
            "a16rep": np.ascontiguousarray(a16rep),
            **shared,
        })
    return in_maps


_NC_CACHE = {}


def _get_nc():
    if "nc" not in _NC_CACHE:
        _NC_CACHE["nc"] = build_nc()
    return _NC_CACHE["nc"]


def kernel_with_results(trace: bool = False, **inputs):
    nc = _get_nc()
    in_maps = prep_host(inputs)
    res = run_bass_kernel_spmd(
        nc, in_maps, core_ids=list(range(N_CORES)), trace=trace,
    )
    out = np.empty((N_FULL, 1), dtype=np.float32)
    for c in range(N_CORES):
        out[c * NSH:(c + 1) * NSH, 0] = res.results[c]["outt"][0]
    return out, res


def kernel(**inputs) -> np.ndarray:
    out, _ = kernel_with_results(trace=False, **inputs)
    return out
